# revision 46
# baseline (speedup 1.0000x reference)
"""Trainium2 Bass kernel: sliding-window multihead attention w/ ALiBi.

Computation (per reference):
  qkv = x @ w_in.T ; q,k,v heads ; blocked sliding-window causal attention
  (window=512, ALiBi bias slope_h*(q_idx-kv_idx)) ; out = o @ w_out.T

Sharding: 8 cores = 4 batches x 2 head-groups (8 heads each). Each core
computes its batch's QKV for its heads, attention, and a partial out-proj
over its heads' columns. Host sums the two head-group partials per batch.

Softmax trick: P = exp(s_raw) * EXPBIG where EXPBIG = exp(bias - bound)
is a host-precomputed Toeplitz band (exact 0 outside the valid window).
The row-max subtraction is replaced by a static bound folded into EXPBIG
(block 0 uses a per-partition ACT bias instead). The softmax denominator
comes from an appended ones-column in the V matmul; normalization uses a
K=2 broadcast matmul + vector reciprocal.
"""

import os
import numpy as np
from contextlib import ExitStack

import concourse.bass as bass
import concourse.bacc as bacc
import concourse.tile as tile
import concourse.mybir as mybir
from concourse.bass_utils import run_bass_kernel_spmd

F16 = mybir.dt.float16
F32 = mybir.dt.float32
AF = mybir.ActivationFunctionType
ALU = mybir.AluOpType

B, S, E = 4, 2048, 1024
H, D, WIN = 16, 64, 512
NB = S // WIN          # 4 blocks
HPC = 8                # heads per core
NCORES = 8
CM = 6.0               # softmax bound safety margin

LAST_RESULTS = None


def _qrange(jt):
    # valid q-column range for scores j-tile jt (window band)
    lo = max(0, 128 * jt - 512)
    hi = min(512, 128 * jt + 128)
    return lo, hi - lo


def _build_nc():
    nc = bacc.Bacc("TRN2", target_bir_lowering=False, debug=False,
                   num_devices=NCORES)

    xT = nc.dram_tensor("xT", [E, S], F16, kind="ExternalInput").ap()
    wqk = nc.dram_tensor("w_qk", [E, 1024], F16, kind="ExternalInput").ap()
    wv = nc.dram_tensor("w_v", [E, 512], F16, kind="ExternalInput").ap()
    wo = nc.dram_tensor("w_o", [512, E], F16, kind="ExternalInput").ap()
    ebig = nc.dram_tensor("expbig", [4, 128, 2816], F16,
                          kind="ExternalInput").ap()
    em0 = nc.dram_tensor("em0", [128, 1024], F16, kind="ExternalInput").ap()
    b0v = nc.dram_tensor("b0v", [128, 32], F32, kind="ExternalInput").ap()
    outp = nc.dram_tensor("out_p", [S, E], F32, kind="ExternalOutput").ap()

    with tile.TileContext(nc) as tc, ExitStack() as ctx:
        pp = ctx.enter_context(tc.tile_pool(name="persist", bufs=1))

        # persistent SBUF tensors
        qkT = [pp.tile([128, S], F16, name=f"qkT{m}", tag=f"qkT{m}")
               for m in range(8)]                       # f-major qk.T
        VA = [pp.tile([128, HPC * 65], F16, name=f"VA{s}", tag=f"VA{s}")
              for s in range(16)]                       # v + ones col per head
        OT = [pp.tile([128, 512], F16, name=f"OT{i}", tag=f"OT{i}")
              for i in range(16)]                       # normalized o.T
        EB = [pp.tile([128, 2816], F16, name=f"EB{h}", tag=f"EB{h}")
              for h in range(4)]                  # exp(bias-bound) band pairs
        EM = pp.tile([128, 1024], F16, name="EM", tag="EM")  # blk0 causal 0/1
        B0 = pp.tile([128, 32], F32, name="B0", tag="B0")    # blk0 exp biases
        WO = [pp.tile([128, E], F16, name=f"WO{k}", tag=f"WO{k}")
              for k in range(4)]

        with tc.tile_pool(name="phA", bufs=1) as pa, \
             tc.tile_pool(name="Pp", bufs=9) as Ppool, \
             tc.tile_pool(name="r2p", bufs=2) as r2p, \
             tc.tile_pool(name="aps", bufs=2, space="PSUM") as aps:
            xTs = [pa.tile([128, S], F16, name=f"xTs{k}", tag=f"xTs{k}")
                   for k in range(8)]
            wqks = [pa.tile([128, 1024], F16, name=f"wqks{k}", tag=f"wqks{k}")
                    for k in range(8)]
            wvs = [pa.tile([128, 512], F16, name=f"wvs{k}", tag=f"wvs{k}")
                   for k in range(8)]
            for k in range(8):
                nc.sync.dma_start(xTs[k][:], xT[128 * k:128 * (k + 1), :])
                nc.sync.dma_start(wqks[k][:], wqk[128 * k:128 * (k + 1), :])
                nc.sync.dma_start(wvs[k][:], wv[128 * k:128 * (k + 1), :])
            for st in range(16):
                nc.gpsimd.memset(VA[st][:], 1.0)

            # ---- projection b chunk: v[s, f] into VA (ones col preserved) --
            def proj_b_chunk(st):
                pv = aps.tile([128, 512], F32, name=f"pv{st}", tag="projch",
                              bufs=2)
                for kt in range(8):
                    nc.tensor.matmul(
                        pv[:],
                        xTs[kt][:, 128 * st:128 * (st + 1)],
                        wvs[kt][:],
                        start=(kt == 0), stop=(kt == 7))
                src = pv.rearrange("p (h c) -> p h c", h=HPC)
                dst = VA[st].rearrange("p (h c) -> p h c", h=HPC)
                nc.scalar.activation(dst[:, :, 0:64], src[:], AF.Copy)

            # ---- projection a: qkT[f, s], one (mt, sc) chunk at a time ----
            def proj_a_chunk(mt, sc):
                ps = aps.tile([128, 512], F32, name=f"pa{mt}_{sc}",
                              tag="projch", bufs=2)
                for kt in range(8):
                    nc.tensor.matmul(
                        ps[:],
                        wqks[kt][:, 128 * mt:128 * (mt + 1)],
                        xTs[kt][:, 512 * sc:512 * (sc + 1)],
                        start=(kt == 0), stop=(kt == 7))
                nc.vector.tensor_copy(qkT[mt][:, 512 * sc:512 * (sc + 1)],
                                      ps[:])

            # ---- out-projection chunk (one s-tile) ----
            def outproj_chunk(st):
                blk_, qq = st // 4, st % 4
                for half in range(2):
                    po = aps.tile([128, 512], F32, name=f"po{st}_{half}",
                                  tag="projch", bufs=2)
                    for kt in range(4):
                        nc.tensor.matmul(
                            po[:],
                            OT[4 * kt + blk_][:, 128 * qq:128 * (qq + 1)],
                            WO[kt][:, 512 * half:512 * (half + 1)],
                            start=(kt == 0), stop=(kt == 3))
                    stg = pa.tile([128, 512], F32, name=f"stg{st}_{half}",
                                  tag="stg", bufs=4)
                    nc.scalar.activation(stg[:], po[:], AF.Copy)
                    nc.sync.dma_start(
                        outp[128 * st:128 * (st + 1),
                             512 * half:512 * (half + 1)], stg[:])

            # serial prefix: just what attention(hp0, blk0/1) needs
            for mt in (0, 4):
                for sc in range(4):
                    proj_a_chunk(mt, sc)
            # constants needed only once attention starts — keep the DMA
            # queues clear for xT/w during the prefix
            for h in range(4):
                nc.sync.dma_start(EB[h][:], ebig[h])
            nc.sync.dma_start(EM[:], em0[:])
            nc.sync.dma_start(B0[:], b0v[:])
            for k in range(4):
                nc.sync.dma_start(WO[k][:], wo[128 * k:128 * (k + 1), :])
            for st in range(8):
                proj_b_chunk(st)

            def attention(hp, fillers):
                for blk in range(4):
                    jts = list(range(8)) if blk > 0 else [4, 5, 6, 7]
                    first_jt = 3 if blk > 0 else 4
                    pv_order = [first_jt] + [j for j in jts if j != first_jt]
                    # paired psum: cols [0:512) head 2hp, [512:1024) head 2hp+1
                    # rows 0-63: o numerator, row 64: denom,
                    # rows 64-127 later overwritten by denom-recip broadcast
                    Op = aps.tile([128, 1024], F32, name=f"O{hp}_{blk}",
                                  tag="Opair", bufs=1)
                    Pt = {}
                    for jt in jts:
                        q0, w = _qrange(jt)
                        gsb = (blk - 1) * 512 + 128 * jt
                        Sp = aps.tile([128, 1024], F32,
                                      name=f"S{hp}_{blk}_{jt}", tag="S")
                        for par in (0, 1):
                            nc.tensor.matmul(
                                Sp[:, 512 * par:512 * par + w],
                                qkT[4 + hp][64 * par:64 * par + 64,
                                            gsb:gsb + 128],
                                qkT[hp][64 * par:64 * par + 64,
                                        512 * blk + q0:512 * blk + q0 + w],
                                start=True, stop=True,
                                tile_position=(64 * par, 0),
                                skip_group_check=True)
                        P = Ppool.tile([128, 1024], F16,
                                       name=f"P{hp}_{blk}_{jt}", tag="P")
                        c0 = q0 - 128 * jt + 896
                        Pv = P.rearrange("p (two c) -> p two c", two=2)
                        Sv = Sp.rearrange("p (two c) -> p two c", two=2)
                        if blk > 0:
                            # paired exp + bias-mul (one op for both heads)
                            nc.scalar.activation(Pv[:, :, 0:w], Sv[:, :, 0:w],
                                                 AF.Exp)
                            EBv = EB[hp].rearrange("p (two c) -> p two c",
                                                   two=2)
                            nc.vector.tensor_tensor(
                                Pv[:, :, 0:w], Pv[:, :, 0:w],
                                EBv[:, :, c0:c0 + w], ALU.mult)
                        else:
                            for par in (0, 1):
                                idx = (2 * hp + par) * 4 + (jt - 4)
                                nc.scalar.activation(
                                    P[:, 512 * par:512 * par + w],
                                    Sp[:, 512 * par:512 * par + w], AF.Exp,
                                    bias=B0[:, idx:idx + 1])
                            EMv = EM.rearrange("p (two c) -> p two c", two=2)
                            nc.gpsimd.tensor_tensor(
                                Pv[:, :, 0:w], Pv[:, :, 0:w],
                                EMv[:, :, 0:w], ALU.mult)
                        Pt[jt] = (P, q0, w)
                    # PE filler while ACT/DVE chew on the exps/muls
                    for kind, arg in fillers[blk]:
                        if kind == 'a':
                            proj_a_chunk(*arg)
                        elif kind == 'b':
                            proj_b_chunk(arg)
                        else:
                            outproj_chunk(arg)
                    for i, jt in enumerate(pv_order):
                        st = 4 * (blk - 1) + jt
                        for par in (0, 1):
                            P, q0, w = Pt[jt]
                            hl = 2 * hp + par
                            nc.tensor.matmul(
                                Op[0:65, 512 * par + q0:512 * par + q0 + w],
                                VA[st][:, 65 * hl:65 * hl + 65],
                                P[:, 512 * par:512 * par + w],
                                start=(i == 0), stop=(i == len(pv_order) - 1),
                                skip_group_check=True)
                    # normalize: reciprocal of denom row, gpsimd broadcast
                    # approx_fast's bitwise path misreads accumulated PSUM
                    # (non-IEEE accumulator bits) — bounce via SBUF first
                    dnc = r2p.tile([1, 1024], F32, name=f"dnc{hp}_{blk}",
                                   tag="dnc")
                    nc.vector.tensor_copy(dnc[:], Op[64:65, :])
                    rr = r2p.tile([1, 1024], F32, name=f"rr{hp}_{blk}",
                                  tag="rr")
                    nc.vector.reciprocal_approx_fast(rr[:], dnc[:])
                    ot = OT[4 * hp + blk]
                    for par in (0, 1):
                        # broadcast 1/denom across partitions on idle gpsimd
                        # (partition_broadcast only writes at partition 0)
                        R2s = r2p.tile([64, 512], F32,
                                       name=f"R2s{hp}_{blk}_{par}",
                                       tag=f"R2s{par}")
                        nc.gpsimd.partition_broadcast(
                            R2s[:], rr[0:1, 512 * par:512 * par + 512])
                        nc.vector.tensor_tensor(
                            ot[64 * par:64 * par + 64, :],
                            Op[0:64, 512 * par:512 * par + 512],
                            R2s[:], ALU.mult)

            # interleave remaining projection + out-projection chunks with
            # attention so scalar/vector engines start early, PE stays warm
            filler_plan = {
                0: [[('b', 8), ('b', 9), ('a', (1, 0))],
                    [('b', 10), ('b', 11), ('a', (1, 1))],
                    [('b', 12), ('b', 13), ('a', (1, 2)), ('a', (1, 3))],
                    [('b', 14), ('b', 15), ('a', (5, 0)), ('a', (5, 1))]],
                1: [[('a', (5, 2)), ('a', (5, 3))],
                    [('a', (2, 0)), ('a', (2, 1))],
                    [('a', (2, 2)), ('a', (2, 3))],
                    [('a', (6, 0)), ('a', (6, 1))]],
                2: [[('a', (6, 2)), ('a', (6, 3))],
                    [('a', (3, 0)), ('a', (3, 1))],
                    [('a', (3, 2)), ('a', (3, 3))],
                    [('a', (7, 0)), ('a', (7, 1))]],
                3: [[('a', (7, 2)), ('a', (7, 3))],
                    [('o', 0), ('o', 1), ('o', 2), ('o', 3)],
                    [('o', 4), ('o', 5), ('o', 6), ('o', 7)],
                    [('o', 8), ('o', 9), ('o', 10), ('o', 11)]],
            }
            for hp in range(4):
                attention(hp, filler_plan[hp])
            for st in range(12, 16):
                outproj_chunk(st)

    nc.compile()
    return nc


_NC = None


def _get_nc():
    global _NC
    if _NC is None:
        _NC = _build_nc()
    return _NC


def _host_consts():
    slopes = np.exp2(-(np.arange(H, dtype=np.float64) + 1.0) * 8.0 / H)
    p = np.arange(128)[:, None]
    c = np.arange(1408)[None, :]
    delta = (c - p - 384).astype(np.float64)
    valid = (delta >= 0) & (delta <= 512)
    eb = np.zeros((H, 128, 1408), np.float16)
    for h in range(H):
        vals = np.exp(slopes[h] * (delta - 512.0) - CM)
        eb[h] = np.where(valid, vals, 0.0).astype(np.float16)
    cc = np.arange(512)[None, :]
    em0 = (cc >= p).astype(np.float16)
    em0 = np.concatenate([em0, em0], axis=1)  # paired [128, 1024]
    # pair-interleaved bands: [g, hp, 128, 2*1408]
    ebp = np.zeros((2, 4, 128, 2816), np.float16)
    for g in range(2):
        for hp in range(4):
            ebp[g, hp, :, 0:1408] = eb[8 * g + 2 * hp]
            ebp[g, hp, :, 1408:2816] = eb[8 * g + 2 * hp + 1]
    b0 = np.zeros((2, 128, 32), np.float32)  # per head-group
    for g in range(2):
        for hl in range(HPC):
            for jtl in range(4):
                b0[g, :, hl * 4 + jtl] = (
                    -slopes[8 * g + hl] * (128.0 * jtl + p[:, 0]) - CM)
    return slopes, ebp, em0, b0


def kernel(x, w_in, w_out):
    global LAST_RESULTS
    x = np.asarray(x, dtype=np.float32)
    w_in = np.asarray(w_in, dtype=np.float32)
    w_out = np.asarray(w_out, dtype=np.float32)

    nc = _get_nc()
    _, ebp, em0, b0 = _host_consts()

    in_maps = []
    for core in range(NCORES):
        b, g = divmod(core, 2)
        r0 = 512 * g
        w_qk = np.ascontiguousarray(np.concatenate(
            [w_in[r0:r0 + 512] * 0.125,
             w_in[E + r0:E + r0 + 512]], axis=0).T).astype(np.float16)
        w_v = np.ascontiguousarray(
            w_in[2 * E + r0:2 * E + r0 + 512].T).astype(np.float16)
        w_o = np.ascontiguousarray(
            w_out[:, r0:r0 + 512].T).astype(np.float16)
        xTc = np.ascontiguousarray(x[b].T).astype(np.float16)
        in_maps.append({
            "xT": xTc,
            "w_qk": w_qk,
            "w_v": w_v,
            "w_o": w_o,
            "expbig": np.ascontiguousarray(ebp[g]),
            "em0": em0,
            "b0v": np.ascontiguousarray(b0[g]),
        })

    res = run_bass_kernel_spmd(nc, in_maps, core_ids=list(range(NCORES)))
    LAST_RESULTS = res
    out = np.stack([
        res.results[2 * b]["out_p"] + res.results[2 * b + 1]["out_p"]
        for b in range(B)
    ]).astype(np.float32)
    return out


# revision 49
# speedup vs baseline: 1.1812x; 1.1812x over previous
"""Trainium2 Bass kernel: sliding-window multihead attention w/ ALiBi.

Computation (per reference):
  qkv = x @ w_in.T ; q,k,v heads ; blocked sliding-window causal attention
  (window=512, ALiBi bias slope_h*(q_idx-kv_idx)) ; out = o @ w_out.T

Sharding: 8 cores = 4 batches x 2 head-groups (8 heads each). Each core
computes its batch's QKV for its heads, attention, and a partial out-proj
over its heads' columns. Host sums the two head-group partials per batch.

Softmax trick: P = exp(s_raw) * EXPBIG where EXPBIG = exp(bias - bound)
is a host-precomputed Toeplitz band (exact 0 outside the valid window).
The row-max subtraction is replaced by a static bound folded into EXPBIG
(block 0 uses a per-partition ACT bias instead). The softmax denominator
comes from an appended ones-column in the V matmul; normalization uses a
K=2 broadcast matmul + vector reciprocal.
"""

import os
import numpy as np
from contextlib import ExitStack

import concourse.bass as bass
import concourse.bacc as bacc
import concourse.tile as tile
import concourse.mybir as mybir
from concourse.bass_utils import run_bass_kernel_spmd

F16 = mybir.dt.float16
F32 = mybir.dt.float32
AF = mybir.ActivationFunctionType
ALU = mybir.AluOpType

B, S, E = 4, 2048, 1024
H, D, WIN = 16, 64, 512
NB = S // WIN          # 4 blocks
HPC = 8                # heads per core
NCORES = 8
CM = 6.0               # softmax bound safety margin

LAST_RESULTS = None


def _qrange(jt):
    # valid q-column range for scores j-tile jt (window band)
    lo = max(0, 128 * jt - 512)
    hi = min(512, 128 * jt + 128)
    return lo, hi - lo


def _build_nc():
    nc = bacc.Bacc("TRN2", target_bir_lowering=False, debug=False,
                   num_devices=NCORES)

    xT = nc.dram_tensor("xT", [E, S], F16, kind="ExternalInput").ap()
    wqk = nc.dram_tensor("w_qk", [E, 1024], F16, kind="ExternalInput").ap()
    wv = nc.dram_tensor("w_v", [E, 512], F16, kind="ExternalInput").ap()
    wo = nc.dram_tensor("w_o", [512, E], F16, kind="ExternalInput").ap()
    ebig = nc.dram_tensor("expbig", [4, 128, 2816], F16,
                          kind="ExternalInput").ap()
    em0 = nc.dram_tensor("em0", [128, 1024], F16, kind="ExternalInput").ap()
    b0v = nc.dram_tensor("b0v", [128, 32], F32, kind="ExternalInput").ap()
    outp = nc.dram_tensor("out_p", [S, E], F32, kind="ExternalOutput").ap()

    with tile.TileContext(nc) as tc, ExitStack() as ctx:
        pp = ctx.enter_context(tc.tile_pool(name="persist", bufs=1))

        # persistent SBUF tensors
        qkT = [pp.tile([128, S], F16, name=f"qkT{m}", tag=f"qkT{m}")
               for m in range(8)]                       # f-major qk.T
        VA = [pp.tile([128, HPC * 65], F16, name=f"VA{s}", tag=f"VA{s}")
              for s in range(16)]                       # v + ones col per head
        OT = [pp.tile([128, 512], F16, name=f"OT{i}", tag=f"OT{i}")
              for i in range(16)]                       # normalized o.T
        EB = [pp.tile([128, 2816], F16, name=f"EB{h}", tag=f"EB{h}")
              for h in range(4)]                  # exp(bias-bound) band pairs
        EM = pp.tile([128, 1024], F16, name="EM", tag="EM")  # blk0 causal 0/1
        B0 = pp.tile([128, 32], F32, name="B0", tag="B0")    # blk0 exp biases
        ONES = pp.tile([1, 64], F16, name="ONES", tag="ONES")
        WO = [pp.tile([128, E], F16, name=f"WO{k}", tag=f"WO{k}")
              for k in range(4)]

        with tc.tile_pool(name="phA", bufs=1) as pa, \
             tc.tile_pool(name="Pp", bufs=9) as Ppool, \
             tc.tile_pool(name="r2p", bufs=2) as r2p, \
             tc.tile_pool(name="aps", bufs=2, space="PSUM") as aps:
            xTs = [pa.tile([128, S], F16, name=f"xTs{k}", tag=f"xTs{k}")
                   for k in range(8)]
            wqks = [pa.tile([128, 1024], F16, name=f"wqks{k}", tag=f"wqks{k}")
                    for k in range(8)]
            wvs = [pa.tile([128, 512], F16, name=f"wvs{k}", tag=f"wvs{k}")
                   for k in range(8)]
            for k in range(8):
                nc.sync.dma_start(xTs[k][:], xT[128 * k:128 * (k + 1), :])
                nc.sync.dma_start(wqks[k][:], wqk[128 * k:128 * (k + 1), :])
                nc.sync.dma_start(wvs[k][:], wv[128 * k:128 * (k + 1), :])
            for st in range(16):
                nc.gpsimd.memset(VA[st][:], 1.0)
            nc.gpsimd.memset(ONES[:], 1.0)

            # ---- projection b chunk: v[s, f] into VA (ones col preserved) --
            def proj_b_chunk(st):
                pv = aps.tile([128, 512], F32, name=f"pv{st}", tag="projch",
                              bufs=2)
                for kt in range(8):
                    nc.tensor.matmul(
                        pv[:],
                        xTs[kt][:, 128 * st:128 * (st + 1)],
                        wvs[kt][:],
                        start=(kt == 0), stop=(kt == 7))
                src = pv.rearrange("p (h c) -> p h c", h=HPC)
                dst = VA[st].rearrange("p (h c) -> p h c", h=HPC)
                nc.scalar.activation(dst[:, :, 0:64], src[:], AF.Copy)

            # ---- projection a: qkT[f, s], one (mt, sc) chunk at a time ----
            def proj_a_chunk(mt, sc):
                ps = aps.tile([128, 512], F32, name=f"pa{mt}_{sc}",
                              tag="projch", bufs=2)
                for kt in range(8):
                    nc.tensor.matmul(
                        ps[:],
                        wqks[kt][:, 128 * mt:128 * (mt + 1)],
                        xTs[kt][:, 512 * sc:512 * (sc + 1)],
                        start=(kt == 0), stop=(kt == 7))
                nc.vector.tensor_copy(qkT[mt][:, 512 * sc:512 * (sc + 1)],
                                      ps[:])

            # ---- out-projection chunk (one s-tile) ----
            def outproj_chunk(st):
                blk_, qq = st // 4, st % 4
                for half in range(2):
                    po = aps.tile([128, 512], F32, name=f"po{st}_{half}",
                                  tag="projch", bufs=2)
                    for kt in range(4):
                        nc.tensor.matmul(
                            po[:],
                            OT[4 * kt + blk_][:, 128 * qq:128 * (qq + 1)],
                            WO[kt][:, 512 * half:512 * (half + 1)],
                            start=(kt == 0), stop=(kt == 3))
                    stg = pa.tile([128, 512], F32, name=f"stg{st}_{half}",
                                  tag="stg", bufs=4)
                    nc.scalar.activation(stg[:], po[:], AF.Copy)
                    nc.sync.dma_start(
                        outp[128 * st:128 * (st + 1),
                             512 * half:512 * (half + 1)], stg[:])

            # serial prefix: just what attention(hp0, blk0/1) needs
            for mt in (0, 4):
                for sc in range(4):
                    proj_a_chunk(mt, sc)
            # constants needed only once attention starts — keep the DMA
            # queues clear for xT/w during the prefix
            for h in range(4):
                nc.sync.dma_start(EB[h][:], ebig[h])
            nc.sync.dma_start(EM[:], em0[:])
            nc.sync.dma_start(B0[:], b0v[:])
            for k in range(4):
                nc.sync.dma_start(WO[k][:], wo[128 * k:128 * (k + 1), :])
            for st in range(8):
                proj_b_chunk(st)

            def attention(hp, fillers):
                for blk in range(4):
                    jts = list(range(8)) if blk > 0 else [4, 5, 6, 7]
                    first_jt = 3 if blk > 0 else 4
                    pv_order = [first_jt] + [j for j in jts if j != first_jt]
                    # paired psum: cols [0:512) head 2hp, [512:1024) head 2hp+1
                    # rows 0-63: o numerator, row 64: denom,
                    # rows 64-127 later overwritten by denom-recip broadcast
                    Op = aps.tile([128, 1024], F32, name=f"O{hp}_{blk}",
                                  tag="Opair", bufs=1)
                    Pt = {}
                    for jt in jts:
                        q0, w = _qrange(jt)
                        gsb = (blk - 1) * 512 + 128 * jt
                        Sp = aps.tile([128, 1024], F32,
                                      name=f"S{hp}_{blk}_{jt}", tag="S")
                        for par in (0, 1):
                            nc.tensor.matmul(
                                Sp[:, 512 * par:512 * par + w],
                                qkT[4 + hp][64 * par:64 * par + 64,
                                            gsb:gsb + 128],
                                qkT[hp][64 * par:64 * par + 64,
                                        512 * blk + q0:512 * blk + q0 + w],
                                start=True, stop=True,
                                tile_position=(64 * par, 0),
                                skip_group_check=True)
                        P = Ppool.tile([128, 1024], F16,
                                       name=f"P{hp}_{blk}_{jt}", tag="P")
                        c0 = q0 - 128 * jt + 896
                        Pv = P.rearrange("p (two c) -> p two c", two=2)
                        Sv = Sp.rearrange("p (two c) -> p two c", two=2)
                        if blk > 0:
                            # paired exp + bias-mul (one op for both heads)
                            nc.scalar.activation(Pv[:, :, 0:w], Sv[:, :, 0:w],
                                                 AF.Exp)
                            EBv = EB[hp].rearrange("p (two c) -> p two c",
                                                   two=2)
                            nc.vector.tensor_tensor(
                                Pv[:, :, 0:w], Pv[:, :, 0:w],
                                EBv[:, :, c0:c0 + w], ALU.mult)
                        else:
                            for par in (0, 1):
                                idx = (2 * hp + par) * 4 + (jt - 4)
                                nc.scalar.activation(
                                    P[:, 512 * par:512 * par + w],
                                    Sp[:, 512 * par:512 * par + w], AF.Exp,
                                    bias=B0[:, idx:idx + 1])
                            EMv = EM.rearrange("p (two c) -> p two c", two=2)
                            nc.gpsimd.tensor_tensor(
                                Pv[:, :, 0:w], Pv[:, :, 0:w],
                                EMv[:, :, 0:w], ALU.mult)
                        Pt[jt] = (P, q0, w)
                    # PE filler while ACT/DVE chew on the exps/muls
                    for kind, arg in fillers[blk]:
                        if kind == 'a':
                            proj_a_chunk(*arg)
                        elif kind == 'b':
                            proj_b_chunk(arg)
                        else:
                            outproj_chunk(arg)
                    for i, jt in enumerate(pv_order):
                        st = 4 * (blk - 1) + jt
                        for par in (0, 1):
                            P, q0, w = Pt[jt]
                            hl = 2 * hp + par
                            nc.tensor.matmul(
                                Op[0:65, 512 * par + q0:512 * par + q0 + w],
                                VA[st][:, 65 * hl:65 * hl + 65],
                                P[:, 512 * par:512 * par + w],
                                start=(i == 0), stop=(i == len(pv_order) - 1),
                                skip_group_check=True)
                    # normalize: reciprocal of denom row, gpsimd broadcast
                    # approx_fast's bitwise path misreads accumulated PSUM
                    # (non-IEEE accumulator bits) — bounce via SBUF first
                    dnc = r2p.tile([1, 1024], F32, name=f"dnc{hp}_{blk}",
                                   tag="dnc")
                    nc.vector.tensor_copy(dnc[:], Op[64:65, :])
                    rr = r2p.tile([1, 1024], F32, name=f"rr{hp}_{blk}",
                                  tag="rr")
                    nc.vector.reciprocal_approx_fast(rr[:], dnc[:])
                    rh = r2p.tile([1, 1024], F16, name=f"rh{hp}_{blk}",
                                  tag="rh")
                    nc.vector.tensor_copy(rh[:], rr[:])
                    # broadcast 1/denom into Op rows 64-127 (K=1 matmuls)
                    for par in (0, 1):
                        nc.tensor.matmul(
                            Op[64:128, 512 * par:512 * par + 512],
                            ONES[0:1, :],
                            rh[0:1, 512 * par:512 * par + 512],
                            start=True, stop=True,
                            tile_position=(0, 64),
                            skip_group_check=True)
                    R2s = r2p.tile([64, 1024], F32, name=f"R2s{hp}_{blk}",
                                   tag="R2s")
                    nc.scalar.activation(R2s[:], Op[64:128, :], AF.Copy)
                    ot = OT[4 * hp + blk]
                    for par in (0, 1):
                        nc.vector.tensor_tensor(
                            ot[64 * par:64 * par + 64, :],
                            Op[0:64, 512 * par:512 * par + 512],
                            R2s[0:64, 512 * par:512 * par + 512], ALU.mult)

            # interleave remaining projection + out-projection chunks with
            # attention so scalar/vector engines start early, PE stays warm
            filler_plan = {
                0: [[('b', 8), ('b', 9), ('a', (1, 0))],
                    [('b', 10), ('b', 11), ('a', (1, 1))],
                    [('b', 12), ('b', 13), ('a', (1, 2)), ('a', (1, 3))],
                    [('b', 14), ('b', 15), ('a', (5, 0)), ('a', (5, 1))]],
                1: [[('a', (5, 2)), ('a', (5, 3))],
                    [('a', (2, 0)), ('a', (2, 1))],
                    [('a', (2, 2)), ('a', (2, 3))],
                    [('a', (6, 0)), ('a', (6, 1))]],
                2: [[('a', (6, 2)), ('a', (6, 3))],
                    [('a', (3, 0)), ('a', (3, 1))],
                    [('a', (3, 2)), ('a', (3, 3))],
                    [('a', (7, 0)), ('a', (7, 1))]],
                3: [[('a', (7, 2)), ('a', (7, 3))],
                    [('o', 0), ('o', 1), ('o', 2), ('o', 3)],
                    [('o', 4), ('o', 5), ('o', 6), ('o', 7)],
                    [('o', 8), ('o', 9), ('o', 10), ('o', 11)]],
            }
            for hp in range(4):
                attention(hp, filler_plan[hp])
            for st in range(12, 16):
                outproj_chunk(st)

    nc.compile()
    return nc


_NC = None


def _get_nc():
    global _NC
    if _NC is None:
        _NC = _build_nc()
    return _NC


def _host_consts():
    slopes = np.exp2(-(np.arange(H, dtype=np.float64) + 1.0) * 8.0 / H)
    p = np.arange(128)[:, None]
    c = np.arange(1408)[None, :]
    delta = (c - p - 384).astype(np.float64)
    valid = (delta >= 0) & (delta <= 512)
    eb = np.zeros((H, 128, 1408), np.float16)
    for h in range(H):
        vals = np.exp(slopes[h] * (delta - 512.0) - CM)
        eb[h] = np.where(valid, vals, 0.0).astype(np.float16)
    cc = np.arange(512)[None, :]
    em0 = (cc >= p).astype(np.float16)
    em0 = np.concatenate([em0, em0], axis=1)  # paired [128, 1024]
    # pair-interleaved bands: [g, hp, 128, 2*1408]
    ebp = np.zeros((2, 4, 128, 2816), np.float16)
    for g in range(2):
        for hp in range(4):
            ebp[g, hp, :, 0:1408] = eb[8 * g + 2 * hp]
            ebp[g, hp, :, 1408:2816] = eb[8 * g + 2 * hp + 1]
    b0 = np.zeros((2, 128, 32), np.float32)  # per head-group
    for g in range(2):
        for hl in range(HPC):
            for jtl in range(4):
                b0[g, :, hl * 4 + jtl] = (
                    -slopes[8 * g + hl] * (128.0 * jtl + p[:, 0]) - CM)
    return slopes, ebp, em0, b0


def kernel(x, w_in, w_out):
    global LAST_RESULTS
    x = np.asarray(x, dtype=np.float32)
    w_in = np.asarray(w_in, dtype=np.float32)
    w_out = np.asarray(w_out, dtype=np.float32)

    nc = _get_nc()
    _, ebp, em0, b0 = _host_consts()

    in_maps = []
    for core in range(NCORES):
        b, g = divmod(core, 2)
        r0 = 512 * g
        w_qk = np.ascontiguousarray(np.concatenate(
            [w_in[r0:r0 + 512] * 0.125,
             w_in[E + r0:E + r0 + 512]], axis=0).T).astype(np.float16)
        w_v = np.ascontiguousarray(
            w_in[2 * E + r0:2 * E + r0 + 512].T).astype(np.float16)
        w_o = np.ascontiguousarray(
            w_out[:, r0:r0 + 512].T).astype(np.float16)
        xTc = np.ascontiguousarray(x[b].T).astype(np.float16)
        in_maps.append({
            "xT": xTc,
            "w_qk": w_qk,
            "w_v": w_v,
            "w_o": w_o,
            "expbig": np.ascontiguousarray(ebp[g]),
            "em0": em0,
            "b0v": np.ascontiguousarray(b0[g]),
        })

    res = run_bass_kernel_spmd(nc, in_maps, core_ids=list(range(NCORES)))
    LAST_RESULTS = res
    out = np.stack([
        res.results[2 * b]["out_p"] + res.results[2 * b + 1]["out_p"]
        for b in range(B)
    ]).astype(np.float32)
    return out
